# revision 2
# baseline (speedup 1.0000x reference)
"""Trainium2 Bass kernel for MultiHeadedAttention with RoPE — v2.

Problem: b=4, n=2048, d=1024, H=16 heads, dk=64, rotary on first 32 dims
(interleaved pairs, theta=10000, lucidrains convention).

Sharding: 8 cores = 4 batches x 2 query-halves (data parallel). Each core
computes the full K/V projections for its batch (replicated across the 2
query-half siblings) and attention + output projection for its 1024 query
rows. No collectives; host gathers/concatenates.

v2 changes vs baseline:
  - RoPE: PSUM drain + bias moves to the (otherwise idle) scalar engine,
    the rotary partner copy is made by SBUF->SBUF DMAs, and the rotate is
    3 full-width DVE ops (mul-cos, mul-sin, add) in bf16 — instead of 6
    DVE ops with 4 narrow 16-partition ops.
  - Attention: heads processed in (even, odd) pairs so the two K=64 score
    matmuls run concurrently in distinct PE row groups; exp alternates
    between the scalar engine (exact, table exp) and the vector engine
    (Schraudolph bitcast exp2 into a bf16 view of an int16 write) so the
    softmax exp — the former single-engine bottleneck — is split across
    two engines; the attnV for iteration kc-1 is emitted after the scores
    of kc so the PE never queues behind the exp of the current iteration.
  - Query halves (qh) are the outer loop; the qh=0 half of the output
    projection is emitted before the qh=1 attention so it fills PE gaps.
  - V phase: per-kc input DMA slices so matmuls start ~15 us earlier.
  - Normalize: reciprocal uses the fast approx DVE op, y = po * (1/Z)
    broadcast via a DRAM-bounce DMA (as baseline) at half-NQ granularity.
"""

import os

import numpy as np

B, N, D = 4, 2048, 1024
H, DK = 16, 64
ROT, HALF = 32, 16
THETA = 10000.0
NCORES = 8
NQ = N // 2  # query rows per core

_PROGRAM_CACHE = {}

# Schraudolph exp2 constants for bf16 bitcast (value = 2^(z) approx):
#   i16 = round(s * C0 + C1); C0 = 128*log2(e)/8, C1 = 128*(127 + sigma)
SCHRAUD_C0 = 128.0 * float(np.log2(np.e)) / 8.0
SCHRAUD_SIGMA = float(os.environ.get("KSIGMA", "0.0430"))
SCHRAUD_C1 = 128.0 * (127.0 + SCHRAUD_SIGMA)

# Of the 16 (qh, pair) groups, this many run BOTH head-halves' exp on the
# scalar engine; the rest split head A -> ACT exact exp, head B -> DVE
# Schraudolph (per-query consistent, so softmax errors stay multiplicative).
N_ALLACT_GROUPS = int(os.environ.get("KEXPACT", "2"))


def _build_program(mm_dtype_name="bfloat16"):
    import concourse.tile as tile
    from concourse import bacc, mybir
    from contextlib import ExitStack

    PHASES = int(os.environ.get("KPHASES", "9"))  # debug bisect knob
    KLOOP = int(os.environ.get("KLOOP", "1"))      # hw-loop repeat (timing)

    f32 = mybir.dt.float32
    i16 = mybir.dt.int16
    mmdt = getattr(mybir.dt, mm_dtype_name)
    AF = mybir.ActivationFunctionType
    ALU = mybir.AluOpType

    nc = bacc.Bacc("TRN2", target_bir_lowering=False)

    # DRAM I/O (per core). All *T tensors are feature-major (transposed).
    xqT = nc.dram_tensor("xqT", [D, NQ], mmdt, kind="ExternalInput")
    xkT = nc.dram_tensor("xkT", [D, N], mmdt, kind="ExternalInput")
    xvT = nc.dram_tensor("xvT", [D, N], mmdt, kind="ExternalInput")
    wqT = nc.dram_tensor("wqT", [D, D], mmdt, kind="ExternalInput")
    wkT = nc.dram_tensor("wkT", [D, D], mmdt, kind="ExternalInput")
    wvT = nc.dram_tensor("wvT", [D, H * 65], mmdt, kind="ExternalInput")
    wvb = nc.dram_tensor("wvb", [1, H * 65], mmdt, kind="ExternalInput")
    woT = nc.dram_tensor("woT", [D, D], mmdt, kind="ExternalInput")
    bq_d = nc.dram_tensor("bq_d", [D], f32, kind="ExternalInput")
    bk_d = nc.dram_tensor("bk_d", [D], f32, kind="ExternalInput")
    bo_d = nc.dram_tensor("bo_d", [D], f32, kind="ExternalInput")
    cosQ = nc.dram_tensor("cosQ", [128, NQ], mmdt, kind="ExternalInput")
    sinQ = nc.dram_tensor("sinQ", [128, NQ], mmdt, kind="ExternalInput")
    cosK = nc.dram_tensor("cosK", [128, N], mmdt, kind="ExternalInput")
    sinK = nc.dram_tensor("sinK", [128, N], mmdt, kind="ExternalInput")
    outT = nc.dram_tensor("outT", [D, NQ], f32, kind="ExternalOutput")

    NKC = N // 128       # 16 key chunks
    NFC = D // 128       # 8 feature chunks
    NVC = D // 128       # 8 contraction chunks for V

    with ExitStack() as ctx:
        tc = ctx.enter_context(tile.TileContext(nc))

        const = ctx.enter_context(tc.tile_pool(name="const", bufs=1))
        dram = ctx.enter_context(tc.tile_pool(name="dram", bufs=4, space="DRAM"))

        # persistent sbuf tensors
        v_sb = const.tile([128, NKC, H * 65], mmdt)   # V' (keys, per-head 64+ones)
        q_sb = const.tile([128, NFC, NQ], mmdt)       # Q_rot.T
        k_sb = const.tile([128, NFC, N], mmdt)        # K_rot.T
        y_sb = const.tile([128, NFC, NQ], mmdt)       # Y.T (normalized attn out)
        bq_sb = const.tile([128, NFC], f32)
        bk_sb = const.tile([128, NFC], f32)
        bo_sb = const.tile([128, NFC], f32)
        cq_sb = const.tile([128, NQ], mmdt)
        sq_sb = const.tile([128, NQ], mmdt)
        ck_sb = const.tile([128, N], mmdt)
        sk_sb = const.tile([128, N], mmdt)
        ones1 = const.tile([1, 128], mmdt)
        wvb_sb = const.tile([1, H * 65], mmdt)
        nc.vector.memset(ones1[:], 1.0)
        nc.sync.dma_start(wvb_sb[:], wvb[:])

        _dmaq = [nc.sync, nc.scalar, nc.gpsimd]
        _dmaqi = [0]

        def dma_rr(dst, src_ap):
            eng = _dmaq[_dmaqi[0] % len(_dmaq)]
            _dmaqi[0] += 1
            eng.dma_start(dst, src_ap)

        def load_chunked(dst_tile, src_t, nchunks, splits=4):
            # dst [128, nchunks, cols]; src (c p) cols layout
            per = nchunks // splits if nchunks % splits == 0 else 1
            if per == 0:
                per = 1
            c = 0
            while c < nchunks:
                n = min(per, nchunks - c)
                dma_rr(
                    dst_tile[:, c:c + n, :],
                    src_t[c * 128:(c + n) * 128, :].rearrange(
                        "(c p) r -> p c r", p=128),
                )
                c += n

        nc.sync.dma_start(bq_sb[:], bq_d.rearrange("(c p) -> p c", p=128))
        nc.sync.dma_start(bk_sb[:], bk_d.rearrange("(c p) -> p c", p=128))
        nc.sync.dma_start(bo_sb[:], bo_d.rearrange("(c p) -> p c", p=128))
        nc.sync.dma_start(cq_sb[:], cosQ[:])
        nc.sync.dma_start(sq_sb[:], sinQ[:])
        nc.sync.dma_start(ck_sb[:], cosK[:])
        nc.sync.dma_start(sk_sb[:], sinK[:])

        def phase_v():
            with tc.tile_pool(name="vphase", bufs=1) as vp, \
                 tc.tile_pool(name="vpsum", bufs=4, space="PSUM") as vps:
                xv_sb = vp.tile([128, NVC, N], mmdt)
                wv_sb = vp.tile([128, NVC, H * 65], mmdt)
                load_chunked(wv_sb, wvT, NVC)
                # per-kc slices of xv so the kc=0 matmuls start early
                for kc in range(NKC):
                    dma_rr(
                        xv_sb[:, :, kc * 128:(kc + 1) * 128],
                        xvT[:, kc * 128:(kc + 1) * 128].rearrange(
                            "(c p) r -> p c r", p=128),
                    )
                for kc in range(NKC):
                    for nf in range(4):  # 1040 = 4 * 260
                        ps = vps.tile([128, 260], f32, tag="vps")
                        for dc in range(NVC):
                            nc.tensor.matmul(
                                ps[:],
                                lhsT=xv_sb[:, dc, kc * 128:(kc + 1) * 128],
                                rhs=wv_sb[:, dc, nf * 260:(nf + 1) * 260],
                                start=(dc == 0),
                                stop=False,
                            )
                        # bias + ones row (K=1): V' gets +bv and the Z column
                        nc.tensor.matmul(
                            ps[:],
                            lhsT=ones1[:, 0:128],
                            rhs=wvb_sb[:, nf * 260:(nf + 1) * 260],
                            start=False,
                            stop=True,
                        )
                        nc.scalar.activation(
                            v_sb[:, kc, nf * 260:(nf + 1) * 260], ps[:], AF.Identity
                        )

        def proj_rope(x_sb, w_sb, b_sb, cos_sb, sin_sb, dst_sb, nrows,
                      rope_pool, rope_psum):
            # dst.T[feat_chunk] over row blocks of 1024
            for fc in range(NFC):
                for rb in range(nrows // 1024):
                    r0 = rb * 1024
                    ps = rope_psum.tile([128, 1024], f32, tag="qk_ps")
                    for dc in range(NFC):
                        for h512 in range(2):
                            nc.tensor.matmul(
                                ps[:, h512 * 512:(h512 + 1) * 512],
                                lhsT=w_sb[:, dc, fc * 128:(fc + 1) * 128],
                                rhs=x_sb[:, dc, r0 + h512 * 512:r0 + (h512 + 1) * 512],
                                start=(dc == 0),
                                stop=(dc == NFC - 1),
                            )
                    # drain + bias on the scalar engine (bf16 out)
                    raw = rope_pool.tile([128, 1024], mmdt, tag="raw")
                    nc.scalar.activation(raw[:], ps[:], AF.Identity,
                                         bias=b_sb[:, fc:fc + 1])
                    # rotary partner copy (+-32 partitions) via SBUF->SBUF DMA
                    swp = rope_pool.tile([128, 1024], mmdt, tag="swp")
                    for d0, s0 in ((0, 32), (32, 0), (64, 96), (96, 64)):
                        dma_rr(swp[d0:d0 + 32, :], raw[s0:s0 + 32, :])
                    # dst = raw*cos + swp*sin   (sin is 0 on pass rows)
                    tmpc = rope_pool.tile([128, 1024], mmdt, tag="tmpc")
                    tmps = rope_pool.tile([128, 1024], mmdt, tag="tmps")
                    nc.vector.tensor_mul(tmpc[:], raw[:], cos_sb[:, r0:r0 + 1024])
                    nc.vector.tensor_mul(tmps[:], swp[:], sin_sb[:, r0:r0 + 1024])
                    nc.vector.tensor_add(
                        dst_sb[:, fc, r0:r0 + 1024], tmpc[:], tmps[:]
                    )

        def phase_q():
            with tc.tile_pool(name="qphase", bufs=1) as qp, \
                 tc.tile_pool(name="qrope", bufs=3) as qrp, \
                 tc.tile_pool(name="qpsum", bufs=2, space="PSUM") as qps:
                xq_sb = qp.tile([128, NFC, NQ], mmdt)
                wq_sb = qp.tile([128, NFC, D], mmdt)
                load_chunked(xq_sb, xqT, NFC)
                load_chunked(wq_sb, wqT, NFC)
                proj_rope(xq_sb, wq_sb, bq_sb, cq_sb, sq_sb, q_sb, NQ,
                          qrp, qps)

        def phase_k():
            with tc.tile_pool(name="kphase", bufs=1) as kp, \
                 tc.tile_pool(name="krope", bufs=3) as krp, \
                 tc.tile_pool(name="kpsum", bufs=2, space="PSUM") as kps:
                xk_sb = kp.tile([128, NFC, N], mmdt)
                wk_sb = kp.tile([128, NFC, D], mmdt)
                load_chunked(xk_sb, xkT, NFC)
                load_chunked(wk_sb, wkT, NFC)
                proj_rope(xk_sb, wk_sb, bk_sb, ck_sb, sk_sb, k_sb, N,
                          krp, kps)

        def phase_attn_out():
            with tc.tile_pool(name="spsum", bufs=2, space="PSUM") as sps, \
                 tc.tile_pool(name="opsum", bufs=2, space="PSUM") as ops_pool, \
                 tc.tile_pool(name="ppool", bufs=3) as pp, \
                 tc.tile_pool(name="npool", bufs=2) as npl, \
                 tc.tile_pool(name="ophase", bufs=1) as op_pool, \
                 tc.tile_pool(name="owork", bufs=2) as owork:
                wo_sb = op_pool.tile([128, NFC, D], mmdt)
                load_chunked(wo_sb, woT, NFC)

                def attn_pair(c, qh):
                    g = qh * NFC + c
                    all_act = ((g * N_ALLACT_GROUPS) % 16) < N_ALLACT_GROUPS
                    hA, hB = 2 * c, 2 * c + 1
                    q0 = qh * 512
                    po_A = ops_pool.tile([65, 512], f32, tag="poA")
                    po_B = ops_pool.tile([65, 512], f32, tag="poB")
                    pend = None

                    def emit_attnv(pt, kc):
                        nc.tensor.matmul(
                            po_A[:],
                            lhsT=v_sb[:, kc, hA * 65:(hA + 1) * 65],
                            rhs=pt[:, 0:512],
                            start=(kc == 0),
                            stop=(kc == NKC - 1),
                        )
                        nc.tensor.matmul(
                            po_B[:],
                            lhsT=v_sb[:, kc, hB * 65:(hB + 1) * 65],
                            rhs=pt[:, 512:1024],
                            start=(kc == 0),
                            stop=(kc == NKC - 1),
                        )

                    for kc in range(NKC):
                        s = sps.tile([128, 1024], f32, tag="s")
                        nc.tensor.matmul(
                            s[:, 0:512],
                            lhsT=k_sb[0:64, c, kc * 128:(kc + 1) * 128],
                            rhs=q_sb[0:64, c, q0:q0 + 512],
                            start=True, stop=True,
                        )
                        nc.tensor.matmul(
                            s[:, 512:1024],
                            lhsT=k_sb[64:128, c, kc * 128:(kc + 1) * 128],
                            rhs=q_sb[64:128, c, q0:q0 + 512],
                            start=True, stop=True,
                        )
                        if pend is not None:
                            emit_attnv(*pend)
                        pt = pp.tile([128, 1024], mmdt, tag="pt")
                        if all_act:
                            nc.scalar.activation(pt[:], s[:], AF.Exp,
                                                 scale=1.0 / 8.0)
                        else:
                            nc.scalar.activation(pt[:, 0:512], s[:, 0:512],
                                                 AF.Exp, scale=1.0 / 8.0)
                            nc.vector.tensor_scalar(
                                pt[:, 512:1024].bitcast(i16), s[:, 512:1024],
                                SCHRAUD_C0, SCHRAUD_C1,
                                op0=ALU.mult, op1=ALU.add,
                            )
                        pend = (pt, kc)
                    emit_attnv(*pend)

                    # normalize: y[head rows] = po[0:64] * (1/Z); the Z row is
                    # broadcast to 64 partitions via a DRAM bounce, then the
                    # reciprocal runs on the SBUF broadcast (the custom DVE
                    # recip op faults when reading PSUM directly).
                    for po, hb in ((po_A, 0), (po_B, 64)):
                        zst = npl.tile([1, 512], f32, tag="zst")
                        nc.any.tensor_copy(zst[:], po[64:65, :])
                        z_dram = dram.tile([1, 512], f32, tag="rzd")
                        nc.sync.dma_start(z_dram[:], zst[:])
                        zb = npl.tile([64, 512], f32, tag="zb")
                        nc.sync.dma_start(
                            zb[:], z_dram[:].to_broadcast([64, 512]))
                        zr = npl.tile([64, 512], f32, tag="zr")
                        nc.vector.reciprocal_approx_fast(zr[:], zb[:])
                        nc.vector.tensor_mul(
                            y_sb[hb:hb + 64, c, q0:q0 + 512], po[0:64, :],
                            zr[:],
                        )

                def outproj_half(qh):
                    q0 = qh * 512
                    for dmc in range(NFC):
                        ps = ops_pool.tile([128, 512], f32, tag="poA")
                        for fc in range(NFC):
                            nc.tensor.matmul(
                                ps[:],
                                lhsT=wo_sb[:, fc, dmc * 128:(dmc + 1) * 128],
                                rhs=y_sb[:, fc, q0:q0 + 512],
                                start=(fc == 0),
                                stop=(fc == NFC - 1),
                            )
                        ob = owork.tile([128, 512], f32, tag="ob")
                        nc.any.tensor_scalar_add(
                            ob[:], ps[:], bo_sb[:, dmc:dmc + 1])
                        dma_rr(outT[dmc * 128:(dmc + 1) * 128, q0:q0 + 512],
                               ob[:])

                for qh in range(2):
                    for c in range(NFC):
                        attn_pair(c, qh)
                    outproj_half(qh)

        def all_phases():
            if PHASES >= 1:
                phase_v()
            if PHASES >= 2:
                phase_q()
            if PHASES >= 3:
                phase_k()
            if PHASES >= 4:
                phase_attn_out()
            else:
                nc.vector.memset(y_sb[:], 0.0)
                with tc.tile_pool(name="dummy", bufs=1) as dp:
                    zb = dp.tile([128, NQ], f32)
                    nc.vector.memset(zb[:], 0.0)
                    for dmc in range(NFC):
                        nc.sync.dma_start(outT[dmc * 128:(dmc + 1) * 128, :], zb[:])

        if KLOOP > 1:
            with tc.For_i(0, KLOOP, 1):
                all_phases()
        else:
            all_phases()

    nc.compile()
    return nc


def _rope_tables(positions):
    """cos/sin tables [128, len(positions)] for the permuted transposed
    layout: partition p (within a 2-head feature chunk), j = p % 64:
    j<16: freq j (cos, -sin); 32<=j<48: freq j-32 (cos, +sin); else (1, 0)."""
    inv_freq = 1.0 / (THETA ** (np.arange(0, ROT, 2, dtype=np.float64) / ROT))  # [16]
    t = np.asarray(positions, dtype=np.float64)
    ang = t[None, :] * inv_freq[:, None]  # [16, nt]
    c, s = np.cos(ang), np.sin(ang)
    cos_tab = np.ones((128, len(positions)), dtype=np.float64)
    sin_tab = np.zeros((128, len(positions)), dtype=np.float64)
    for h2 in (0, 64):
        cos_tab[h2:h2 + 16] = c
        cos_tab[h2 + 32:h2 + 48] = c
        sin_tab[h2:h2 + 16] = -s
        sin_tab[h2 + 32:h2 + 48] = s
    return cos_tab.astype(np.float32), sin_tab.astype(np.float32)


def _head_perm():
    """Feature permutation applied to rows of Wq/Wk (and bq/bk): within each
    head's 64 outputs -> [evens(16), pass 32:48, odds(16), pass 48:64]."""
    out = np.empty(D, dtype=np.int64)
    for h in range(H):
        base = h * DK
        out[base:base + HALF] = base + np.arange(0, ROT, 2)
        out[base + HALF:base + ROT] = base + np.arange(ROT, ROT + HALF)
        out[base + ROT:base + ROT + HALF] = base + np.arange(1, ROT, 2)
        out[base + ROT + HALF:base + DK] = base + np.arange(ROT + HALF, DK)
    return out


def _prep_inputs(query, key, value, Wq, bq, Wk, bk, Wv, bv, Wo, bo,
                 mm_dtype_name="bfloat16"):
    import ml_dtypes

    np_mm = ml_dtypes.bfloat16 if mm_dtype_name == "bfloat16" else np.float32

    query = np.asarray(query, np.float32)
    key = np.asarray(key, np.float32)
    value = np.asarray(value, np.float32)
    Wq, bq = np.asarray(Wq, np.float32), np.asarray(bq, np.float32)
    Wk, bk = np.asarray(Wk, np.float32), np.asarray(bk, np.float32)
    Wv, bv = np.asarray(Wv, np.float32), np.asarray(bv, np.float32)
    Wo, bo = np.asarray(Wo, np.float32), np.asarray(bo, np.float32)

    perm = _head_perm()
    Wq_p, bq_p = Wq[perm], bq[perm]
    Wk_p, bk_p = Wk[perm], bk[perm]

    wqT = np.ascontiguousarray(Wq_p.T).astype(np_mm)
    wkT = np.ascontiguousarray(Wk_p.T).astype(np_mm)
    woT = np.ascontiguousarray(Wo.T).astype(np_mm)

    # W_v' : [D, H*65] plus a separate bias/ones row wvb [1, H*65]
    wvT = np.zeros((D, H * 65), np.float32)
    wvb = np.zeros((1, H * 65), np.float32)
    for h in range(H):
        cols = slice(h * 65, h * 65 + 64)
        wvT[:D, cols] = Wv[h * DK:(h + 1) * DK, :].T
        wvb[0, cols] = bv[h * DK:(h + 1) * DK]
        wvb[0, h * 65 + 64] = 1.0
    wvT = wvT.astype(np_mm)
    wvb = wvb.astype(np_mm)

    cos_all, sin_all = _rope_tables(np.arange(N))

    in_maps = []
    for core in range(NCORES):
        b, qh = core // 2, core % 2
        rows = slice(qh * NQ, (qh + 1) * NQ)
        xqT = np.ascontiguousarray(query[b, rows, :].T).astype(np_mm)
        xkT = np.ascontiguousarray(key[b].T).astype(np_mm)
        xvT = np.ascontiguousarray(value[b].T).astype(np_mm)
        in_maps.append({
            "xqT": xqT,
            "xkT": xkT,
            "xvT": xvT,
            "wqT": wqT, "wkT": wkT, "wvT": wvT, "woT": woT, "wvb": wvb,
            "bq_d": bq_p, "bk_d": bk_p, "bo_d": bo,
            "cosQ": np.ascontiguousarray(cos_all[:, rows]).astype(np_mm),
            "sinQ": np.ascontiguousarray(sin_all[:, rows]).astype(np_mm),
            "cosK": cos_all.astype(np_mm),
            "sinK": sin_all.astype(np_mm),
        })
    return in_maps


def kernel(query, key, value, Wq, bq, Wk, bk, Wv, bv, Wo, bo):
    from concourse import bass_utils

    mm_dtype_name = "bfloat16"
    if mm_dtype_name not in _PROGRAM_CACHE:
        _PROGRAM_CACHE[mm_dtype_name] = _build_program(mm_dtype_name)
    nc = _PROGRAM_CACHE[mm_dtype_name]

    in_maps = _prep_inputs(query, key, value, Wq, bq, Wk, bk, Wv, bv, Wo, bo,
                           mm_dtype_name)

    res = bass_utils.run_bass_kernel_spmd(
        nc, in_maps, core_ids=list(range(NCORES))
    )

    out = np.empty((B, N, D), np.float32)
    for core in range(NCORES):
        b, qh = core // 2, core % 2
        out[b, qh * NQ:(qh + 1) * NQ, :] = res.results[core]["outT"].T
    return out
